# revision 8
# baseline (speedup 1.0000x reference)
"""Trainium2 Bass kernel for a pre-LN causal-attention transformer block, v2.

Sharding: data-parallel over batch. B=64 -> 8 batches per NeuronCore,
processed in 4 chunks of 2 batches (512 tokens).

Design notes:
  - rstd = exp(-0.5*ln(var+eps)) on ACT, and the activation-table list fed to
    the table-load pass is filtered so Ln/Exp resolve to the combined
    natural_log_exp_and_others set -> one table load for the whole program.
  - Attention computed transposed: scoresT[s,t] = kT^T qT directly (kT
    stationary), two heads per score psum tile; causal mask added only on the
    diagonal blocks with a head-broadcast mask; exp writes probsT; attnV uses
    probsT as stationary with an appended ones-column on V giving row sums for
    free; normalization is a per-token reciprocal + broadcast multiply on the
    token-major o psum.
  - All six big GEMMs (Q,K,V,proj,MLP1,MLP2) run in fp8 e4m3 with DoubleRow
    perf mode. Weights are host-scaled by S=32 before quantization; the
    descale rides the psum->sbuf copies. attnV is fp8+DR too. The bp bias is
    a K=1 matmul row appended to the proj psum group.
  - Elementwise/copy work is spread across ACT/DVE/Pool (Pool = Q7 software,
    ~0.5x throughput: gets the lightest share).
"""
import numpy as np
import ml_dtypes

import concourse.tile as tile
import concourse.bacc as bacc_mod
from concourse import bacc, mybir
from concourse.bass_utils import run_bass_kernel_spmd
from concourse.hw_specs import get_activation_tables

F32 = mybir.dt.float32
BF16 = mybir.dt.bfloat16
FP8 = mybir.dt.float8e4
AF = mybir.ActivationFunctionType
ALU = mybir.AluOpType
PM = mybir.MatmulPerfMode

B, T, C = 64, 256, 384
H, HD = 6, 64
FF = 4 * C                      # 1536
NCORES = 8
BL = B // NCORES                # 8 batches per core
TOK = BL * T                    # 2048 tokens per core
CHB = 2                        # batches per chunk
NCH = BL // CHB                 # 4 chunks
CTOK = CHB * T                  # 512 tokens per chunk
NTT = CTOK // 128               # 4 token tiles per chunk
EPS = 1e-5
NEG = -30.0                     # additive causal mask value

F8 = True                       # fp8 + DoubleRow on the big GEMMs
F8A = True                      # fp8 + DoubleRow on attnV probs/V
S = 32.0 if F8 else 1.0         # host weight scale before fp8 quantization
WDT = FP8 if F8 else BF16       # dtype of quantized weights + fm activations
ADT = FP8 if F8A else BF16      # dtype of probs / V

# engine assignment knobs: 'a'=ACT, 'd'=DVE, 'p'=Pool
CFG = dict(
    tp_e0="add", tp_e1="a",       # transpose psum->sbuf copies (1024/512 cols)
    qk_eng="aa",                # qT, kT copies
    v_eng="a",                  # V copies
    mask_eng="d",             # per head-pair: p=affine_select, d=TT-mult
    onorm_eng="d",              # o normalize broadcast-mult
    norm_eng="pp",              # LN normalize halves (SBUF only: p ok)
    relu_eng="ada",              # MLP1 relu cycle (PSUM: a/d only)
    proj_eng="d",               # proj residual stt
    mlp2_eng="d",               # MLP2 residual stt
    pipeline=True,              # software-pipeline chunks
)

_CACHE = {}

_COMBINED_TABLE = "natural_log_exp_and_others"


def _patched_tables(arch):
    """Strip Ln/Exp from every act-func set except the combined one, so the
    table-load pass resolves both to a single resident table."""
    tabs = dict(get_activation_tables(arch))
    ln_exp = {AF.Ln, AF.Exp}
    out = {}
    for name, funcs in tabs.items():
        if name == _COMBINED_TABLE:
            out[name] = funcs
        else:
            out[name] = funcs - ln_exp
    return out


def _build(debug=False, repeat=1):
    nc = bacc.Bacc("TRN2", target_bir_lowering=False, debug=False,
                   num_devices=NCORES)

    # ---- DRAM I/O ----------------------------------------------------------
    x_d = nc.dram_tensor("x", [TOK, C], F32, kind="ExternalInput").ap()
    wq_d = nc.dram_tensor("wq", [128, 3 * C], WDT, kind="ExternalInput").ap()
    wk_d = nc.dram_tensor("wk", [128, 3 * C], WDT, kind="ExternalInput").ap()
    wv_d = nc.dram_tensor("wv", [128, 3 * C], WDT, kind="ExternalInput").ap()
    wp_d = nc.dram_tensor("wp", [128, 3 * C], WDT, kind="ExternalInput").ap()
    w1_d = nc.dram_tensor("w1", [128, 3 * FF], WDT, kind="ExternalInput").ap()
    w2_d = nc.dram_tensor("w2", [128, 12 * C], WDT, kind="ExternalInput").ap()
    cqk_d = nc.dram_tensor("cqk", [128, 6], F32, kind="ExternalInput").ap()
    c1_d = nc.dram_tensor("c1", [128, 12], F32, kind="ExternalInput").ap()
    bpr_d = nc.dram_tensor("bpr", [1, C], WDT, kind="ExternalInput").ap()
    idb_d = nc.dram_tensor("idb", [128, 128], BF16, kind="ExternalInput").ap()
    cmt_d = nc.dram_tensor("cmt", [128, 2 * 128], F32, kind="ExternalInput").ap()
    out_d = nc.dram_tensor("out", [TOK, C], F32, kind="ExternalOutput").ap()
    dbg = {}
    if debug:
        for nm, shape, dt in [
            ("d_h1", [128, NTT, C], BF16), ("d_h1T", [128, 3, CTOK], WDT),
            ("d_qT", [128, 3, CTOK], BF16), ("d_kT", [128, 3, CTOK], BF16),
            ("d_v", [128, NTT, 512], ADT), ("d_pT", [128, 2, 384], ADT),
            ("d_osb", [128, NTT, C], BF16), ("d_x2", [128, NTT, C], F32),
            ("d_a1T", [128, 12, CTOK], WDT),
        ]:
            dbg[nm] = nc.dram_tensor(nm, shape, dt, kind="ExternalOutput").ap()

    with tile.TileContext(nc) as tc:
        with (
            tc.tile_pool(name="const", bufs=1) as cp,
            tc.tile_pool(name="io", bufs=2) as iop,
            tc.tile_pool(name="act", bufs=2) as ap_,
            tc.tile_pool(name="probs", bufs=4) as ptp,
            tc.tile_pool(name="small", bufs=2) as smp,
            tc.tile_pool(name="tpps", bufs=2, space="PSUM") as tpp,
            tc.tile_pool(name="mmps", bufs=2, space="PSUM") as mmp,
            tc.tile_pool(name="scps", bufs=2, space="PSUM") as scp,
        ):
            # ---- persistent weights / constants ---------------------------
            idb_s = cp.tile([128, 128], BF16)
            nc.sync.dma_start(idb_s[:], idb_d[:])
            eps_s = cp.tile([128, 1], F32)
            nc.gpsimd.memset(eps_s[:], EPS)
            # warm the PE clock (pstate ramps to full after ~3us busy) while
            # the first x chunk is still in flight on DMA
            for r in range(2):
                warm = tpp.tile([128, 2, CTOK], BF16, tag="tpA")
                for i in range(8):
                    nc.tensor.matmul(
                        warm[:, i // 4, 128 * (i % 4):128 * (i % 4 + 1)],
                        idb_s[:], idb_s[:], is_transpose=True,
                        start=(i == 0), stop=(i == 7))
            one_s = cp.tile([1, 128], WDT)
            nc.gpsimd.memset(one_s[:], 1.0)
            x_first = iop.tile([128, NTT, C], F32, tag="x")
            nc.sync.dma_start(
                x_first[:],
                x_d[0:CTOK, :].rearrange("(t p) c -> p t c", p=128))
            wq_s = cp.tile([128, 3, C], WDT)
            wk_s = cp.tile([128, 3, C], WDT)
            nc.sync.dma_start(wq_s[:].rearrange("p k c -> p (k c)"), wq_d[:])
            nc.sync.dma_start(wk_s[:].rearrange("p k c -> p (k c)"), wk_d[:])
            cqk_s = cp.tile([128, 6], F32)
            nc.sync.dma_start(cqk_s[:], cqk_d[:])
            wv_s = cp.tile([128, 3, C], WDT)
            nc.sync.dma_start(wv_s[:].rearrange("p k c -> p (k c)"), wv_d[:])
            cmt_s = cp.tile([128, 2, 128], F32)
            nc.sync.dma_start(cmt_s[:].rearrange("p a b -> p (a b)"), cmt_d[:])
            wp_s = cp.tile([128, 3, C], WDT)
            nc.sync.dma_start(wp_s[:].rearrange("p k c -> p (k c)"), wp_d[:])
            bpr_s = cp.tile([1, C], WDT)
            nc.sync.dma_start(bpr_s[:], bpr_d[:])
            w1_s = cp.tile([128, 3, FF], WDT)
            nc.sync.dma_start(w1_s[:].rearrange("p k c -> p (k c)"), w1_d[:])
            c1_s = cp.tile([128, 12], F32)
            nc.sync.dma_start(c1_s[:], c1_d[:])
            w2_s = cp.tile([128, 12, C], WDT)
            nc.sync.dma_start(w2_s[:].rearrange("p k c -> p (k c)"), w2_d[:])

            def load_x(ch):
                base = (ch % NCH) * CTOK
                t = iop.tile([128, NTT, C], F32, tag="x")
                nc.sync.dma_start(
                    t[:], x_d[base:base + CTOK, :].rearrange(
                        "(t p) c -> p t c", p=128))
                return t

            ENG = {"a": nc.scalar, "d": nc.vector, "p": nc.gpsimd}

            def layernorm2(src, tag):
                """token-major LN -> normalized bf16 tile [128, NTT, C].

                sum(x) on DVE, sum(x^2) via ACT Square accum (DVE is the
                bottleneck engine; ACT has headroom), small var ops on DVE,
                rstd = exp(-0.5 ln(var+eps)) on ACT, normalize per CFG.
                """
                s1 = smp.tile([128, NTT], F32, tag=f"{tag}s1")
                s2 = smp.tile([128, NTT], F32, tag=f"{tag}s2")
                sq = ap_.tile([128, C], F32, tag="sq")
                for tt in range(NTT):
                    nc.vector.reduce_sum(s1[:, tt:tt + 1], src[:, tt],
                                         axis=mybir.AxisListType.X)
                    nc.scalar.activation(sq[:], src[:, tt], AF.Square,
                                         bias=0.0, scale=1.0,
                                         accum_out=s2[:, tt:tt + 1])
                mu = smp.tile([128, NTT], F32, tag=f"{tag}mu")
                nc.vector.tensor_scalar_mul(mu[:], s1[:], 1.0 / C)
                musq = smp.tile([128, NTT], F32, tag=f"{tag}mq")
                nc.vector.tensor_tensor(musq[:], mu[:], mu[:], op=ALU.mult)
                var = smp.tile([128, NTT], F32, tag=f"{tag}va")
                nc.vector.scalar_tensor_tensor(
                    var[:], s2[:], 1.0 / C, musq[:],
                    op0=ALU.mult, op1=ALU.subtract)
                lnv = smp.tile([128, NTT], F32, tag=f"{tag}lv")
                nc.scalar.activation(lnv[:], var[:], AF.Ln,
                                     bias=eps_s[:], scale=1.0)
                rstd = smp.tile([128, NTT], F32, tag=f"{tag}rs")
                nc.scalar.activation(rstd[:], lnv[:], AF.Exp,
                                     bias=0.0, scale=-0.5)
                h = ap_.tile([128, NTT, C], BF16, tag=f"{tag}h")
                for tt in range(NTT):
                    eng = ENG[CFG["norm_eng"][tt % len(CFG["norm_eng"])]]
                    eng.tensor_scalar(
                        h[:, tt], src[:, tt],
                        mu[:, tt:tt + 1], rstd[:, tt:tt + 1],
                        op0=ALU.subtract, op1=ALU.mult)
                return h

            TP_SITE = {"h1": 0, "o": 1, "h2": 2}

            def transpose_ft(h, dst_dt, tag):
                """[128, NTT, C] token-major -> [128, 3, CTOK] feature-major."""
                hT = ap_.tile([128, 3, CTOK], dst_dt, tag=f"{tag}T")
                site = TP_SITE[tag]
                e0 = ENG[CFG["tp_e0"][site % len(CFG["tp_e0"])]]
                e1 = ENG[CFG["tp_e1"][site % len(CFG["tp_e1"])]]

                def cpy(eng, dst, src):
                    if eng is nc.scalar:
                        eng.copy(dst, src)
                    else:
                        eng.tensor_copy(dst, src)

                tpA = tpp.tile([128, 2, CTOK], BF16, tag="tpA")
                n = 2 * NTT
                i = 0
                for c in range(2):
                    for tt in range(NTT):
                        nc.tensor.matmul(
                            tpA[:, c, 128 * tt:128 * (tt + 1)],
                            h[:, tt, 128 * c:128 * (c + 1)], idb_s[:],
                            is_transpose=True, start=(i == 0), stop=(i == n - 1))
                        i += 1
                cpy(e0, hT[:, 0:2, :], tpA[:])
                tpB = tpp.tile([128, 2, CTOK], BF16, tag="tpA")
                for tt in range(NTT):
                    nc.tensor.matmul(
                        tpB[:, 0, 128 * tt:128 * (tt + 1)],
                        h[:, tt, 256:384], idb_s[:],
                        is_transpose=True, start=(tt == 0), stop=(tt == NTT - 1))
                cpy(e1, hT[:, 2, :], tpB[:, 0, :])
                return hT

            def mm_k3(ps, lhs_of, rhs_of, fp8):
                """psum = sum_k lhsT_k rhs_k over 3 k-tiles (DR pair + plain),
                leaves the accumulation group open (stop on caller)."""
                if fp8:
                    nc.tensor.matmul(ps, lhs_of(0, 2), rhs_of(0, 2),
                                     start=True, stop=False,
                                     perf_mode=PM.DoubleRow)
                    return lambda stop, extra=None: nc.tensor.matmul(
                        ps, lhs_of(2, 3), rhs_of(2, 3),
                        start=False, stop=stop)
                else:
                    for c in range(2):
                        nc.tensor.matmul(ps, lhs_of(c, c + 1), rhs_of(c, c + 1),
                                         start=(c == 0), stop=False)
                    return lambda stop, extra=None: nc.tensor.matmul(
                        ps, lhs_of(2, 3), rhs_of(2, 3),
                        start=False, stop=stop)

            def sl(t, k0, k1, c0=None, c1=None):
                return t[:, k0, c0:c1] if k1 == k0 + 1 else t[:, k0:k1, c0:c1]

            def front_qkv(st, ch, debug_ch):
                """h1T transposes + Q/K/V projections for one chunk (S2)."""
                # ---- h1 (bf16) -> h1T (WDT, feature-major) -----------------
                h1 = st["h1"]
                h1T = transpose_ft(h1, WDT, "h1")
                if debug and ch == debug_ch:
                    nc.sync.dma_start(dbg["d_h1"][:], h1[:])
                    nc.sync.dma_start(dbg["d_h1T"][:], h1T[:])

                # ---- Q^T, K^T (bf16, bias folded, descale by 1/S) ----------
                qT = ap_.tile([128, 3, CTOK], BF16, tag="qT")
                kT = ap_.tile([128, 3, CTOK], BF16, tag="kT")
                for w_s, oT, bcol, eng in ((wq_s, qT, 0, CFG["qk_eng"][0]),
                                           (wk_s, kT, 3, CFG["qk_eng"][1])):
                    for m in range(3):
                        ps = mmp.tile([128, CTOK], F32, tag="mm")
                        fin = mm_k3(
                            ps[:],
                            lambda k0, k1: sl(w_s, k0, k1, 128 * m, 128 * (m + 1)),
                            lambda k0, k1: sl(h1T, k0, k1), F8)
                        fin(True)
                        if eng == "d":
                            nc.vector.tensor_scalar(
                                oT[:, m], ps[:], 1.0 / S,
                                cqk_s[:, bcol + m:bcol + m + 1],
                                op0=ALU.mult, op1=ALU.add)
                        else:
                            nc.scalar.activation(
                                oT[:, m], ps[:], AF.Identity,
                                bias=cqk_s[:, bcol + m:bcol + m + 1],
                                scale=1.0 / S)

                # ---- V (token-major, ADT, with ones cols for row sums) -----
                v_sb = ap_.tile([128, NTT, 512], ADT, tag="v")
                nc.gpsimd.memset(v_sb[:, :, 64::65], 1.0)

                def v_tile(tt):
                    ps = mmp.tile([128, CTOK], F32, tag="mm")
                    fin = mm_k3(
                        ps[:, 0:C],
                        lambda k0, k1: sl(h1T, k0, k1, 128 * tt, 128 * (tt + 1)),
                        lambda k0, k1: sl(wv_s, k0, k1), F8)
                    fin(True)
                    dst = v_sb[:, tt, 0:390].rearrange(
                        "p (h e) -> p h e", h=6)[:, :, 0:64]
                    src = ps[:, 0:C].rearrange("p (h e) -> p h e", h=6)
                    if CFG["v_eng"] == "a":
                        nc.scalar.activation(dst, src, AF.Identity,
                                             bias=0.0, scale=1.0 / S)
                    else:
                        ENG[CFG["v_eng"]].tensor_scalar_mul(dst, src, 1.0 / S)

                v_tile(0)
                v_tile(1)
                if debug and ch == debug_ch:
                    nc.sync.dma_start(dbg["d_qT"][:], qT[:])
                    nc.sync.dma_start(dbg["d_kT"][:], kT[:])
                st.update(qT=qT, kT=kT, v=v_sb, v_tile=v_tile)

            def attn(st, ch, debug_ch):
                """attention (transposed scores), per batch in chunk (S3).

                Two heads per score-psum tile (2 banks); post-exp causal mask;
                attnV issues after the pair's exp, software-pipelined."""
                qT, kT, v_sb, v_tile = st["qT"], st["kT"], st["v"], st["v_tile"]
                o_sb = ap_.tile([128, NTT, C], BF16, tag="osb")
                for bb in range(CHB):
                    t0 = 2 * bb
                    pTs = []
                    for hp in range(3):         # head pairs (2hp, 2hp+1)
                        sc = scp.tile([128, 2, CTOK], F32, tag="sc")
                        for i in range(2):
                            h = 2 * hp + i
                            qp = 64 * (h % 2)
                            qm = h // 2
                            nc.tensor.matmul(
                                sc[:, i, 0:256],
                                kT[qp:qp + 64, qm, 128 * t0:128 * (t0 + 1)],
                                qT[qp:qp + 64, qm, 128 * t0:128 * (t0 + 2)],
                                start=True, stop=False)
                            nc.tensor.matmul(
                                sc[:, i, 256:384],
                                kT[qp:qp + 64, qm,
                                   128 * (t0 + 1):128 * (t0 + 2)],
                                qT[qp:qp + 64, qm,
                                   128 * (t0 + 1):128 * (t0 + 2)],
                                start=False, stop=True)
                        pT = ptp.tile([128, 2, 384], ADT, tag="pT")
                        nc.scalar.activation(pT[:], sc[:, :, 0:384],
                                             AF.Exp, bias=0.0, scale=1.0)
                        # causal mask: zero probs where s > t on the two
                        # diagonal blocks (iota = t - s, keep iota >= 0)
                        for i in range(2):
                            dvw = pT[:, i].rearrange(
                                "p (a b) -> p a b", b=128)[:, 0::2, :]
                            meng = CFG["mask_eng"][
                                (2 * hp + i) % len(CFG["mask_eng"])]
                            if meng == "p":
                                nc.gpsimd.affine_select(
                                    dvw, dvw, pattern=[[0, 2], [1, 128]],
                                    compare_op=ALU.is_ge, fill=0.0,
                                    base=0, channel_multiplier=-1)
                            else:
                                ENG[meng].tensor_tensor(
                                    dvw, dvw, cmt_s[:], op=ALU.mult)
                        pTs.append(pT)
                        if debug and ch == debug_ch and bb == 0 and hp == 0:
                            nc.sync.dma_start(dbg["d_pT"][:], pT[:])

                    if bb == 0:
                        v_tile(2)
                        v_tile(3)
                    op0 = mmp.tile([128, CTOK], F32, tag="mm")
                    op1 = mmp.tile([128, CTOK], F32, tag="mm")
                    ov0 = op0[:, 0:390].rearrange("p (h e) -> p h e", h=6)
                    ov1 = op1[:, 0:390].rearrange("p (h e) -> p h e", h=6)
                    for h in range(H):
                        pT = pTs[h // 2][:, h % 2]
                        # attnV: t0 tile plain (K=s0), t1 tile DR (K=s0+s1)
                        nc.tensor.matmul(
                            ov0[:, h], pT[:, 0:128],
                            v_sb[:, t0, 65 * h:65 * (h + 1)],
                            start=(h == 0), stop=(h == H - 1))
                        if F8A:
                            nc.tensor.matmul(
                                ov1[:, h],
                                pT[:, 128:384].rearrange(
                                    "p (k t) -> p k t", k=2),
                                v_sb[:, t0:t0 + 2, 65 * h:65 * (h + 1)],
                                start=(h == 0), stop=(h == H - 1),
                                perf_mode=PM.DoubleRow)
                        else:
                            nc.tensor.matmul(
                                ov1[:, h], pT[:, 128:256],
                                v_sb[:, t0, 65 * h:65 * (h + 1)],
                                start=(h == 0), stop=False)
                            nc.tensor.matmul(
                                ov1[:, h], pT[:, 256:384],
                                v_sb[:, t0 + 1, 65 * h:65 * (h + 1)],
                                start=False, stop=(h == H - 1))

                    for tt, ov in ((t0, ov0), (t0 + 1, ov1)):
                        rec = smp.tile([128, 6], F32, tag="rec")
                        nc.vector.reciprocal(rec[:], ov[:, :, 64])
                        ENG[CFG["onorm_eng"]].tensor_tensor(
                            o_sb[:, tt].rearrange("p (h e) -> p h e", h=6),
                            ov[:, :, 0:64],
                            rec[:, :, None].broadcast_to([128, 6, 64]),
                            op=ALU.mult)

                if debug and ch == debug_ch:
                    nc.sync.dma_start(dbg["d_osb"][:], o_sb[:])
                st["o_sb"] = o_sb

            def projx(st, ch, debug_ch):
                """oT transpose + proj + residual + bp -> x2 (S4)."""
                x_sb, o_sb = st["x"], st["o_sb"]
                oT = transpose_ft(o_sb, WDT, "o")
                x2 = ap_.tile([128, NTT, C], F32, tag="x2")
                for tt in range(NTT):
                    ps = mmp.tile([128, CTOK], F32, tag="mm")
                    fin = mm_k3(
                        ps[:, 0:C],
                        lambda k0, k1: sl(oT, k0, k1, 128 * tt, 128 * (tt + 1)),
                        lambda k0, k1: sl(wp_s, k0, k1), F8)
                    fin(False)
                    nc.tensor.matmul(ps[:, 0:C], one_s[:], bpr_s[:],
                                     start=False, stop=True)
                    ENG[CFG["proj_eng"]].scalar_tensor_tensor(
                        x2[:, tt], ps[:, 0:C], 1.0 / S, x_sb[:, tt],
                        op0=ALU.mult, op1=ALU.add)
                if debug and ch == debug_ch:
                    nc.sync.dma_start(dbg["d_x2"][:], x2[:])

                st["x2"] = x2

            def ln2p(st):
                st["h2"] = layernorm2(st["x2"], "l2")

            def mlp1(st, ch, debug_ch):
                h2T = transpose_ft(st["h2"], WDT, "h2")
                a1T = ap_.tile([128, 12, CTOK], WDT, tag="a1T")
                for fm in range(12):
                    ps = mmp.tile([128, CTOK], F32, tag="mm")
                    fin = mm_k3(
                        ps[:],
                        lambda k0, k1: sl(w1_s, k0, k1, 128 * fm, 128 * (fm + 1)),
                        lambda k0, k1: sl(h2T, k0, k1), F8)
                    fin(True)
                    r = CFG["relu_eng"][fm % len(CFG["relu_eng"])]
                    if r == "a":
                        nc.scalar.activation(
                            a1T[:, fm], ps[:], AF.Relu,
                            bias=c1_s[:, fm:fm + 1], scale=1.0)
                    else:
                        ENG[r].tensor_scalar(
                            a1T[:, fm], ps[:], c1_s[:, fm:fm + 1], 0.0,
                            op0=ALU.add, op1=ALU.max)
                if debug and ch == debug_ch:
                    nc.sync.dma_start(dbg["d_a1T"][:], a1T[:])
                st["a1T"] = a1T

            def ln1(st):
                st["h1"] = layernorm2(st["x"], "l1")

            def mlp2(st, ch):
                base = (ch % NCH) * CTOK
                x2, a1T = st["x2"], st["a1T"]
                o_out = iop.tile([128, NTT, C], F32, tag="o")
                for tt in range(NTT):
                    ps = mmp.tile([128, CTOK], F32, tag="mm")
                    if F8:
                        for j in range(6):
                            nc.tensor.matmul(
                                ps[:, 0:C],
                                a1T[:, 2 * j:2 * j + 2, 128 * tt:128 * (tt + 1)],
                                w2_s[:, 2 * j:2 * j + 2, :],
                                start=(j == 0), stop=(j == 5),
                                perf_mode=PM.DoubleRow)
                    else:
                        for j in range(12):
                            nc.tensor.matmul(
                                ps[:, 0:C],
                                a1T[:, j, 128 * tt:128 * (tt + 1)],
                                w2_s[:, j, :],
                                start=(j == 0), stop=(j == 11))
                    ENG[CFG["mlp2_eng"]].scalar_tensor_tensor(
                        o_out[:, tt], ps[:, 0:C], 1.0 / (S * S), x2[:, tt],
                        op0=ALU.mult, op1=ALU.add)
                nc.sync.dma_start(
                    out_d[base:base + CTOK, :].rearrange("(t p) c -> p t c", p=128),
                    o_out[:])

            # ---- software-pipelined chunk loop -------------------------
            # steady state per iteration i (chunk ch = i):
            #   LN1(i+1) | ATTN(i) | PROJ(i) | LN2(i) | QKV(i+1) | MLP(i)
            # so chunk i+1's LN runs during chunk i's attention era and its
            # PE-heavy QKV fills the LN2(i) latency before MLP(i).
            N = NCH * repeat
            dch = 0
            cur = {"x": x_first}
            ln1(cur)
            front_qkv(cur, 0, dch)
            for i in range(N):
                nxt = None
                if i + 1 < N:
                    nxt = {"x": load_x(i + 1)}
                    ln1(nxt)
                attn(cur, i % NCH, dch)
                projx(cur, i % NCH, dch)
                ln2p(cur)
                if nxt is not None:
                    front_qkv(nxt, (i + 1) % NCH, dch)
                mlp1(cur, i % NCH, dch)
                mlp2(cur, i)
                cur = nxt

    # Route the act-table pass through the filtered table list.
    orig = bacc_mod.get_activation_tables
    bacc_mod.get_activation_tables = _patched_tables
    try:
        nc.compile()
    finally:
        bacc_mod.get_activation_tables = orig
    return nc


def _prep_inputs(x, wq, wk, wv, wp, bp, w1, w2, g1, b1, g2, b2):
    """Host-side weight folding + per-core input maps."""
    f32 = np.float32
    qdt = ml_dtypes.float8_e4m3 if F8 else ml_dtypes.bfloat16
    adt = ml_dtypes.float8_e4m3 if F8A else ml_dtypes.bfloat16
    scale = HD ** -0.5
    wq_m = np.ascontiguousarray(np.asarray(wq).transpose(1, 0, 2).reshape(C, C))
    wk_m = np.ascontiguousarray(np.asarray(wk).transpose(1, 0, 2).reshape(C, C))
    wv_m = np.ascontiguousarray(np.asarray(wv).transpose(1, 0, 2).reshape(C, C))
    wq_p = (g1[:, None] * wq_m * scale).astype(f32)
    cq = (b1 @ wq_m * scale).astype(f32)
    wk_p = (g1[:, None] * wk_m).astype(f32)
    ck = (b1 @ wk_m).astype(f32)
    wv_p = (g1[:, None] * wv_m).astype(f32)
    cv = (b1 @ wv_m).astype(f32)
    w1_p = (g2[:, None] * np.asarray(w1)).astype(f32)
    c1 = (b2 @ w1).astype(f32)
    bp_p = (np.asarray(bp, f32) + cv @ np.asarray(wp, f32)).astype(f32)

    def ktile(w, kt):   # [K, M] -> [128, kt*M] k-tile-blocked, quantized
        K, M = w.shape
        return np.ascontiguousarray(
            (w * S).reshape(kt, 128, M).transpose(1, 0, 2).reshape(128, kt * M)
        ).astype(qdt)

    wq8 = ktile(wq_p, 3)
    wk8 = ktile(wk_p, 3)
    wv8 = ktile(wv_p, 3)
    wp8 = ktile(np.asarray(wp, f32), 3)
    w18 = ktile(w1_p, 3)
    w28 = ktile(np.asarray(w2, f32), 12)
    bpr = (S * bp_p)[None, :].astype(qdt)

    cqk = np.concatenate([cq.reshape(3, 128).T, ck.reshape(3, 128).T], axis=1)
    cqk = np.ascontiguousarray(cqk, dtype=f32)                       # [128, 6]
    c1t = np.ascontiguousarray((S * c1).reshape(12, 128).T, dtype=f32)
    idb = np.eye(128).astype(ml_dtypes.bfloat16)
    ii, jj = np.arange(128)[:, None], np.arange(128)[None, :]
    triT = np.where(ii > jj, 0.0, 1.0).astype(f32)    # mult-mask, s > t -> 0
    cmt = np.concatenate([triT, triT], axis=1)

    common = dict(wq=wq8, wk=wk8, wv=wv8, wp=wp8, w1=w18, w2=w28,
                  cqk=cqk, c1=c1t, bpr=bpr, idb=idb, cmt=cmt)
    in_maps = []
    for core in range(NCORES):
        xs = np.ascontiguousarray(
            np.asarray(x)[BL * core:BL * (core + 1)].reshape(TOK, C), dtype=f32)
        in_maps.append(dict(common, x=xs))
    return in_maps


def run(inputs, trace=False, trace_kwargs=None, debug=False):
    key = "nc_dbg" if debug else "nc"
    if key not in _CACHE:
        _CACHE[key] = _build(debug=debug)
    nc = _CACHE[key]
    in_maps = _prep_inputs(**inputs)
    res = run_bass_kernel_spmd(nc, in_maps, list(range(NCORES)),
                               trace=trace, **(trace_kwargs or {}))
    out = np.empty((B, T, C), dtype=np.float32)
    for core in range(NCORES):
        out[BL * core:BL * (core + 1)] = \
            res.results[core]["out"].reshape(BL, T, C)
    return out, res


def kernel(**inputs):
    out, _ = run(inputs)
    return out


# revision 9
# speedup vs baseline: 1.0817x; 1.0817x over previous
"""Trainium2 Bass kernel for a pre-LN causal-attention transformer block, v2.

Sharding: data-parallel over batch. B=64 -> 8 batches per NeuronCore,
processed in 4 chunks of 2 batches (512 tokens).

Design notes:
  - rstd = exp(-0.5*ln(var+eps)) on ACT, and the activation-table list fed to
    the table-load pass is filtered so Ln/Exp resolve to the combined
    natural_log_exp_and_others set -> one table load for the whole program.
  - Attention computed transposed: scoresT[s,t] = kT^T qT directly (kT
    stationary), two heads per score psum tile; causal mask added only on the
    diagonal blocks with a head-broadcast mask; exp writes probsT; attnV uses
    probsT as stationary with an appended ones-column on V giving row sums for
    free; normalization is a per-token reciprocal + broadcast multiply on the
    token-major o psum.
  - All six big GEMMs (Q,K,V,proj,MLP1,MLP2) run in fp8 e4m3 with DoubleRow
    perf mode. Weights are host-scaled by S=32 before quantization; the
    descale rides the psum->sbuf copies. attnV is fp8+DR too. The bp bias is
    a K=1 matmul row appended to the proj psum group.
  - Elementwise/copy work is spread across ACT/DVE/Pool (Pool = Q7 software,
    ~0.5x throughput: gets the lightest share).
"""
import numpy as np
import ml_dtypes

import concourse.tile as tile
import concourse.bacc as bacc_mod
from concourse import bacc, mybir
from concourse.bass_utils import run_bass_kernel_spmd
from concourse.hw_specs import get_activation_tables

F32 = mybir.dt.float32
BF16 = mybir.dt.bfloat16
FP8 = mybir.dt.float8e4
AF = mybir.ActivationFunctionType
ALU = mybir.AluOpType
PM = mybir.MatmulPerfMode

B, T, C = 64, 256, 384
H, HD = 6, 64
FF = 4 * C                      # 1536
NCORES = 8
BL = B // NCORES                # 8 batches per core
TOK = BL * T                    # 2048 tokens per core
CHB = 2                        # batches per chunk
NCH = BL // CHB                 # 4 chunks
CTOK = CHB * T                  # 512 tokens per chunk
NTT = CTOK // 128               # 4 token tiles per chunk
EPS = 1e-5
NEG = -30.0                     # additive causal mask value

F8 = True                       # fp8 + DoubleRow on the big GEMMs
F8A = True                      # fp8 + DoubleRow on attnV probs/V
S = 32.0 if F8 else 1.0         # host weight scale before fp8 quantization
WDT = FP8 if F8 else BF16       # dtype of quantized weights + fm activations
ADT = FP8 if F8A else BF16      # dtype of probs / V

# engine assignment knobs: 'a'=ACT, 'd'=DVE, 'p'=Pool
CFG = dict(
    tp_e0="add", tp_e1="a",       # transpose psum->sbuf copies (1024/512 cols)
    qk_eng="aa",                # qT, kT copies
    v_eng="a",                  # V copies
    mask_eng="d",             # per head-pair: p=affine_select, d=TT-mult
    onorm_eng="d",              # o normalize broadcast-mult
    norm_eng="pp",              # LN normalize halves (SBUF only: p ok)
    relu_eng="ada",              # MLP1 relu cycle (PSUM: a/d only)
    proj_eng="d",               # proj residual stt
    mlp2_eng="d",               # MLP2 residual stt
    pipeline=True,              # software-pipeline chunks
)

_CACHE = {}

_COMBINED_TABLE = "natural_log_exp_and_others"


def _patched_tables(arch):
    """Strip Ln/Exp from every act-func set except the combined one, so the
    table-load pass resolves both to a single resident table."""
    tabs = dict(get_activation_tables(arch))
    ln_exp = {AF.Ln, AF.Exp}
    out = {}
    for name, funcs in tabs.items():
        if name == _COMBINED_TABLE:
            out[name] = funcs
        else:
            out[name] = funcs - ln_exp
    return out


def _build(debug=False, repeat=1):
    nc = bacc.Bacc("TRN2", target_bir_lowering=False, debug=False,
                   num_devices=NCORES)

    # ---- DRAM I/O ----------------------------------------------------------
    x_d = nc.dram_tensor("x", [TOK, C], F32, kind="ExternalInput").ap()
    wq_d = nc.dram_tensor("wq", [128, 3 * C], WDT, kind="ExternalInput").ap()
    wk_d = nc.dram_tensor("wk", [128, 3 * C], WDT, kind="ExternalInput").ap()
    wv_d = nc.dram_tensor("wv", [128, 3 * C], WDT, kind="ExternalInput").ap()
    wp_d = nc.dram_tensor("wp", [128, 3 * C], WDT, kind="ExternalInput").ap()
    w1_d = nc.dram_tensor("w1", [128, 3 * FF], WDT, kind="ExternalInput").ap()
    w2_d = nc.dram_tensor("w2", [128, 12 * C], WDT, kind="ExternalInput").ap()
    cqk_d = nc.dram_tensor("cqk", [128, 6], F32, kind="ExternalInput").ap()
    c1_d = nc.dram_tensor("c1", [128, 12], F32, kind="ExternalInput").ap()
    bpr_d = nc.dram_tensor("bpr", [1, C], WDT, kind="ExternalInput").ap()
    idb_d = nc.dram_tensor("idb", [128, 128], BF16, kind="ExternalInput").ap()
    cmt_d = nc.dram_tensor("cmt", [128, 2 * 128], F32, kind="ExternalInput").ap()
    out_d = nc.dram_tensor("out", [TOK, C], F32, kind="ExternalOutput").ap()
    dbg = {}
    if debug:
        for nm, shape, dt in [
            ("d_h1", [128, NTT, C], BF16), ("d_h1T", [128, 3, CTOK], WDT),
            ("d_qT", [128, 3, CTOK], BF16), ("d_kT", [128, 3, CTOK], BF16),
            ("d_v", [128, NTT, 512], ADT), ("d_pT", [128, 2, 384], ADT),
            ("d_osb", [128, NTT, C], BF16), ("d_x2", [128, NTT, C], F32),
            ("d_a1T", [128, 12, CTOK], WDT),
        ]:
            dbg[nm] = nc.dram_tensor(nm, shape, dt, kind="ExternalOutput").ap()

    with tile.TileContext(nc) as tc:
        with (
            tc.tile_pool(name="const", bufs=1) as cp,
            tc.tile_pool(name="io", bufs=2) as iop,
            tc.tile_pool(name="act", bufs=2) as ap_,
            tc.tile_pool(name="probs", bufs=4) as ptp,
            tc.tile_pool(name="small", bufs=2) as smp,
            tc.tile_pool(name="tpps", bufs=2, space="PSUM") as tpp,
            tc.tile_pool(name="mmps", bufs=2, space="PSUM") as mmp,
            tc.tile_pool(name="scps", bufs=2, space="PSUM") as scp,
        ):
            # ---- persistent weights / constants ---------------------------
            idb_s = cp.tile([128, 128], BF16)
            nc.sync.dma_start(idb_s[:], idb_d[:])
            eps_s = cp.tile([128, 1], F32)
            nc.gpsimd.memset(eps_s[:], EPS)
            one_s = cp.tile([1, 128], WDT)
            nc.gpsimd.memset(one_s[:], 1.0)
            x_first = iop.tile([128, NTT, C], F32, tag="x")
            nc.sync.dma_start(
                x_first[:],
                x_d[0:CTOK, :].rearrange("(t p) c -> p t c", p=128))
            wq_s = cp.tile([128, 3, C], WDT)
            wk_s = cp.tile([128, 3, C], WDT)
            nc.sync.dma_start(wq_s[:].rearrange("p k c -> p (k c)"), wq_d[:])
            nc.sync.dma_start(wk_s[:].rearrange("p k c -> p (k c)"), wk_d[:])
            cqk_s = cp.tile([128, 6], F32)
            nc.sync.dma_start(cqk_s[:], cqk_d[:])
            wv_s = cp.tile([128, 3, C], WDT)
            nc.sync.dma_start(wv_s[:].rearrange("p k c -> p (k c)"), wv_d[:])
            cmt_s = cp.tile([128, 2, 128], F32)
            nc.sync.dma_start(cmt_s[:].rearrange("p a b -> p (a b)"), cmt_d[:])
            wp_s = cp.tile([128, 3, C], WDT)
            nc.sync.dma_start(wp_s[:].rearrange("p k c -> p (k c)"), wp_d[:])
            bpr_s = cp.tile([1, C], WDT)
            nc.sync.dma_start(bpr_s[:], bpr_d[:])
            w1_s = cp.tile([128, 3, FF], WDT)
            nc.sync.dma_start(w1_s[:].rearrange("p k c -> p (k c)"), w1_d[:])
            c1_s = cp.tile([128, 12], F32)
            nc.sync.dma_start(c1_s[:], c1_d[:])
            w2_s = cp.tile([128, 12, C], WDT)
            nc.sync.dma_start(w2_s[:].rearrange("p k c -> p (k c)"), w2_d[:])

            def load_x(ch):
                base = (ch % NCH) * CTOK
                t = iop.tile([128, NTT, C], F32, tag="x")
                nc.sync.dma_start(
                    t[:], x_d[base:base + CTOK, :].rearrange(
                        "(t p) c -> p t c", p=128))
                return t

            ENG = {"a": nc.scalar, "d": nc.vector, "p": nc.gpsimd}

            def layernorm2(src, tag):
                """token-major LN -> normalized bf16 tile [128, NTT, C].

                stats via bn_stats/bn_aggr (DVE), rstd = exp(-0.5 ln(var+eps))
                on ACT, normalize on engines from CFG['norm_eng'].
                """
                st6 = smp.tile([128, NTT, 6], F32, tag=f"{tag}s6")
                msig = smp.tile([128, NTT, 2], F32, tag=f"{tag}ms")
                for tt in range(NTT):
                    nc.vector.bn_stats(st6[:, tt], src[:, tt])
                    nc.vector.bn_aggr(msig[:, tt], st6[:, tt])
                lnv = smp.tile([128, NTT], F32, tag=f"{tag}lv")
                nc.scalar.activation(lnv[:], msig[:, :, 1], AF.Ln,
                                     bias=eps_s[:], scale=1.0)
                rstd = smp.tile([128, NTT], F32, tag=f"{tag}rs")
                nc.scalar.activation(rstd[:], lnv[:], AF.Exp,
                                     bias=0.0, scale=-0.5)
                h = ap_.tile([128, NTT, C], BF16, tag=f"{tag}h")
                for tt in range(NTT):
                    eng = ENG[CFG["norm_eng"][tt % len(CFG["norm_eng"])]]
                    eng.tensor_scalar(
                        h[:, tt], src[:, tt],
                        msig[:, tt, 0:1], rstd[:, tt:tt + 1],
                        op0=ALU.subtract, op1=ALU.mult)
                return h

            TP_SITE = {"h1": 0, "o": 1, "h2": 2}

            def transpose_ft(h, dst_dt, tag):
                """[128, NTT, C] token-major -> [128, 3, CTOK] feature-major."""
                hT = ap_.tile([128, 3, CTOK], dst_dt, tag=f"{tag}T")
                site = TP_SITE[tag]
                e0 = ENG[CFG["tp_e0"][site % len(CFG["tp_e0"])]]
                e1 = ENG[CFG["tp_e1"][site % len(CFG["tp_e1"])]]

                def cpy(eng, dst, src):
                    if eng is nc.scalar:
                        eng.copy(dst, src)
                    else:
                        eng.tensor_copy(dst, src)

                tpA = tpp.tile([128, 2, CTOK], BF16, tag="tpA")
                n = 2 * NTT
                i = 0
                for c in range(2):
                    for tt in range(NTT):
                        nc.tensor.matmul(
                            tpA[:, c, 128 * tt:128 * (tt + 1)],
                            h[:, tt, 128 * c:128 * (c + 1)], idb_s[:],
                            is_transpose=True, start=(i == 0), stop=(i == n - 1))
                        i += 1
                cpy(e0, hT[:, 0:2, :], tpA[:])
                tpB = tpp.tile([128, 2, CTOK], BF16, tag="tpA")
                for tt in range(NTT):
                    nc.tensor.matmul(
                        tpB[:, 0, 128 * tt:128 * (tt + 1)],
                        h[:, tt, 256:384], idb_s[:],
                        is_transpose=True, start=(tt == 0), stop=(tt == NTT - 1))
                cpy(e1, hT[:, 2, :], tpB[:, 0, :])
                return hT

            def mm_k3(ps, lhs_of, rhs_of, fp8):
                """psum = sum_k lhsT_k rhs_k over 3 k-tiles (DR pair + plain),
                leaves the accumulation group open (stop on caller)."""
                if fp8:
                    nc.tensor.matmul(ps, lhs_of(0, 2), rhs_of(0, 2),
                                     start=True, stop=False,
                                     perf_mode=PM.DoubleRow)
                    return lambda stop, extra=None: nc.tensor.matmul(
                        ps, lhs_of(2, 3), rhs_of(2, 3),
                        start=False, stop=stop)
                else:
                    for c in range(2):
                        nc.tensor.matmul(ps, lhs_of(c, c + 1), rhs_of(c, c + 1),
                                         start=(c == 0), stop=False)
                    return lambda stop, extra=None: nc.tensor.matmul(
                        ps, lhs_of(2, 3), rhs_of(2, 3),
                        start=False, stop=stop)

            def sl(t, k0, k1, c0=None, c1=None):
                return t[:, k0, c0:c1] if k1 == k0 + 1 else t[:, k0:k1, c0:c1]

            def front_qkv(st, ch, debug_ch):
                """h1T transposes + Q/K/V projections for one chunk (S2)."""
                # ---- h1 (bf16) -> h1T (WDT, feature-major) -----------------
                h1 = st["h1"]
                h1T = transpose_ft(h1, WDT, "h1")
                if debug and ch == debug_ch:
                    nc.sync.dma_start(dbg["d_h1"][:], h1[:])
                    nc.sync.dma_start(dbg["d_h1T"][:], h1T[:])

                # ---- Q^T, K^T (bf16, bias folded, descale by 1/S) ----------
                qT = ap_.tile([128, 3, CTOK], BF16, tag="qT")
                kT = ap_.tile([128, 3, CTOK], BF16, tag="kT")
                for w_s, oT, bcol, eng in ((wq_s, qT, 0, CFG["qk_eng"][0]),
                                           (wk_s, kT, 3, CFG["qk_eng"][1])):
                    for m in range(3):
                        ps = mmp.tile([128, CTOK], F32, tag="mm")
                        fin = mm_k3(
                            ps[:],
                            lambda k0, k1: sl(w_s, k0, k1, 128 * m, 128 * (m + 1)),
                            lambda k0, k1: sl(h1T, k0, k1), F8)
                        fin(True)
                        if eng == "d":
                            nc.vector.tensor_scalar(
                                oT[:, m], ps[:], 1.0 / S,
                                cqk_s[:, bcol + m:bcol + m + 1],
                                op0=ALU.mult, op1=ALU.add)
                        else:
                            nc.scalar.activation(
                                oT[:, m], ps[:], AF.Identity,
                                bias=cqk_s[:, bcol + m:bcol + m + 1],
                                scale=1.0 / S)

                # ---- V (token-major, ADT, with ones cols for row sums) -----
                v_sb = ap_.tile([128, NTT, 512], ADT, tag="v")
                nc.gpsimd.memset(v_sb[:, :, 64::65], 1.0)

                def v_tile(tt):
                    ps = mmp.tile([128, CTOK], F32, tag="mm")
                    fin = mm_k3(
                        ps[:, 0:C],
                        lambda k0, k1: sl(h1T, k0, k1, 128 * tt, 128 * (tt + 1)),
                        lambda k0, k1: sl(wv_s, k0, k1), F8)
                    fin(True)
                    dst = v_sb[:, tt, 0:390].rearrange(
                        "p (h e) -> p h e", h=6)[:, :, 0:64]
                    src = ps[:, 0:C].rearrange("p (h e) -> p h e", h=6)
                    if CFG["v_eng"] == "a":
                        nc.scalar.activation(dst, src, AF.Identity,
                                             bias=0.0, scale=1.0 / S)
                    else:
                        ENG[CFG["v_eng"]].tensor_scalar_mul(dst, src, 1.0 / S)

                v_tile(0)
                v_tile(1)
                if debug and ch == debug_ch:
                    nc.sync.dma_start(dbg["d_qT"][:], qT[:])
                    nc.sync.dma_start(dbg["d_kT"][:], kT[:])
                st.update(qT=qT, kT=kT, v=v_sb, v_tile=v_tile)

            def attn(st, ch, debug_ch):
                """attention (transposed scores), per batch in chunk (S3).

                Two heads per score-psum tile (2 banks); post-exp causal mask;
                attnV issues after the pair's exp, software-pipelined."""
                qT, kT, v_sb, v_tile = st["qT"], st["kT"], st["v"], st["v_tile"]
                o_sb = ap_.tile([128, NTT, C], BF16, tag="osb")
                for bb in range(CHB):
                    t0 = 2 * bb
                    pTs = []
                    for hp in range(3):         # head pairs (2hp, 2hp+1)
                        sc = scp.tile([128, 2, CTOK], F32, tag="sc")
                        for i in range(2):
                            h = 2 * hp + i
                            qp = 64 * (h % 2)
                            qm = h // 2
                            nc.tensor.matmul(
                                sc[:, i, 0:256],
                                kT[qp:qp + 64, qm, 128 * t0:128 * (t0 + 1)],
                                qT[qp:qp + 64, qm, 128 * t0:128 * (t0 + 2)],
                                start=True, stop=False)
                            nc.tensor.matmul(
                                sc[:, i, 256:384],
                                kT[qp:qp + 64, qm,
                                   128 * (t0 + 1):128 * (t0 + 2)],
                                qT[qp:qp + 64, qm,
                                   128 * (t0 + 1):128 * (t0 + 2)],
                                start=False, stop=True)
                        pT = ptp.tile([128, 2, 384], ADT, tag="pT")
                        nc.scalar.activation(pT[:], sc[:, :, 0:384],
                                             AF.Exp, bias=0.0, scale=1.0)
                        # causal mask: zero probs where s > t on the two
                        # diagonal blocks (iota = t - s, keep iota >= 0)
                        for i in range(2):
                            dvw = pT[:, i].rearrange(
                                "p (a b) -> p a b", b=128)[:, 0::2, :]
                            meng = CFG["mask_eng"][
                                (2 * hp + i) % len(CFG["mask_eng"])]
                            if meng == "p":
                                nc.gpsimd.affine_select(
                                    dvw, dvw, pattern=[[0, 2], [1, 128]],
                                    compare_op=ALU.is_ge, fill=0.0,
                                    base=0, channel_multiplier=-1)
                            else:
                                ENG[meng].tensor_tensor(
                                    dvw, dvw, cmt_s[:], op=ALU.mult)
                        pTs.append(pT)
                        if debug and ch == debug_ch and bb == 0 and hp == 0:
                            nc.sync.dma_start(dbg["d_pT"][:], pT[:])

                    if bb == 0:
                        v_tile(2)
                        v_tile(3)
                    op0 = mmp.tile([128, CTOK], F32, tag="mm")
                    op1 = mmp.tile([128, CTOK], F32, tag="mm")
                    ov0 = op0[:, 0:390].rearrange("p (h e) -> p h e", h=6)
                    ov1 = op1[:, 0:390].rearrange("p (h e) -> p h e", h=6)
                    for h in range(H):
                        pT = pTs[h // 2][:, h % 2]
                        # attnV: t0 tile plain (K=s0), t1 tile DR (K=s0+s1)
                        nc.tensor.matmul(
                            ov0[:, h], pT[:, 0:128],
                            v_sb[:, t0, 65 * h:65 * (h + 1)],
                            start=(h == 0), stop=(h == H - 1))
                        if F8A:
                            nc.tensor.matmul(
                                ov1[:, h],
                                pT[:, 128:384].rearrange(
                                    "p (k t) -> p k t", k=2),
                                v_sb[:, t0:t0 + 2, 65 * h:65 * (h + 1)],
                                start=(h == 0), stop=(h == H - 1),
                                perf_mode=PM.DoubleRow)
                        else:
                            nc.tensor.matmul(
                                ov1[:, h], pT[:, 128:256],
                                v_sb[:, t0, 65 * h:65 * (h + 1)],
                                start=(h == 0), stop=False)
                            nc.tensor.matmul(
                                ov1[:, h], pT[:, 256:384],
                                v_sb[:, t0 + 1, 65 * h:65 * (h + 1)],
                                start=False, stop=(h == H - 1))

                    for tt, ov in ((t0, ov0), (t0 + 1, ov1)):
                        rec = smp.tile([128, 6], F32, tag="rec")
                        nc.vector.reciprocal(rec[:], ov[:, :, 64])
                        ENG[CFG["onorm_eng"]].tensor_tensor(
                            o_sb[:, tt].rearrange("p (h e) -> p h e", h=6),
                            ov[:, :, 0:64],
                            rec[:, :, None].broadcast_to([128, 6, 64]),
                            op=ALU.mult)

                if debug and ch == debug_ch:
                    nc.sync.dma_start(dbg["d_osb"][:], o_sb[:])
                st["o_sb"] = o_sb

            def projx(st, ch, debug_ch):
                """oT transpose + proj + residual + bp -> x2 (S4)."""
                x_sb, o_sb = st["x"], st["o_sb"]
                oT = transpose_ft(o_sb, WDT, "o")
                x2 = ap_.tile([128, NTT, C], F32, tag="x2")
                for tt in range(NTT):
                    ps = mmp.tile([128, CTOK], F32, tag="mm")
                    fin = mm_k3(
                        ps[:, 0:C],
                        lambda k0, k1: sl(oT, k0, k1, 128 * tt, 128 * (tt + 1)),
                        lambda k0, k1: sl(wp_s, k0, k1), F8)
                    fin(False)
                    nc.tensor.matmul(ps[:, 0:C], one_s[:], bpr_s[:],
                                     start=False, stop=True)
                    ENG[CFG["proj_eng"]].scalar_tensor_tensor(
                        x2[:, tt], ps[:, 0:C], 1.0 / S, x_sb[:, tt],
                        op0=ALU.mult, op1=ALU.add)
                if debug and ch == debug_ch:
                    nc.sync.dma_start(dbg["d_x2"][:], x2[:])

                st["x2"] = x2

            def ln2p(st):
                st["h2"] = layernorm2(st["x2"], "l2")

            def mlp1(st, ch, debug_ch):
                h2T = transpose_ft(st["h2"], WDT, "h2")
                a1T = ap_.tile([128, 12, CTOK], WDT, tag="a1T")
                for fm in range(12):
                    ps = mmp.tile([128, CTOK], F32, tag="mm")
                    fin = mm_k3(
                        ps[:],
                        lambda k0, k1: sl(w1_s, k0, k1, 128 * fm, 128 * (fm + 1)),
                        lambda k0, k1: sl(h2T, k0, k1), F8)
                    fin(True)
                    r = CFG["relu_eng"][fm % len(CFG["relu_eng"])]
                    if r == "a":
                        nc.scalar.activation(
                            a1T[:, fm], ps[:], AF.Relu,
                            bias=c1_s[:, fm:fm + 1], scale=1.0)
                    else:
                        ENG[r].tensor_scalar(
                            a1T[:, fm], ps[:], c1_s[:, fm:fm + 1], 0.0,
                            op0=ALU.add, op1=ALU.max)
                if debug and ch == debug_ch:
                    nc.sync.dma_start(dbg["d_a1T"][:], a1T[:])
                st["a1T"] = a1T

            def ln1(st):
                st["h1"] = layernorm2(st["x"], "l1")

            def mlp2(st, ch):
                base = (ch % NCH) * CTOK
                x2, a1T = st["x2"], st["a1T"]
                o_out = iop.tile([128, NTT, C], F32, tag="o")
                for tt in range(NTT):
                    ps = mmp.tile([128, CTOK], F32, tag="mm")
                    if F8:
                        for j in range(6):
                            nc.tensor.matmul(
                                ps[:, 0:C],
                                a1T[:, 2 * j:2 * j + 2, 128 * tt:128 * (tt + 1)],
                                w2_s[:, 2 * j:2 * j + 2, :],
                                start=(j == 0), stop=(j == 5),
                                perf_mode=PM.DoubleRow)
                    else:
                        for j in range(12):
                            nc.tensor.matmul(
                                ps[:, 0:C],
                                a1T[:, j, 128 * tt:128 * (tt + 1)],
                                w2_s[:, j, :],
                                start=(j == 0), stop=(j == 11))
                    ENG[CFG["mlp2_eng"]].scalar_tensor_tensor(
                        o_out[:, tt], ps[:, 0:C], 1.0 / (S * S), x2[:, tt],
                        op0=ALU.mult, op1=ALU.add)
                nc.sync.dma_start(
                    out_d[base:base + CTOK, :].rearrange("(t p) c -> p t c", p=128),
                    o_out[:])

            # ---- software-pipelined chunk loop -------------------------
            # steady state per iteration i (chunk ch = i):
            #   LN1(i+1) | ATTN(i) | PROJ(i) | LN2(i) | QKV(i+1) | MLP(i)
            # so chunk i+1's LN runs during chunk i's attention era and its
            # PE-heavy QKV fills the LN2(i) latency before MLP(i).
            N = NCH * repeat
            dch = 0
            cur = {"x": x_first}
            ln1(cur)
            front_qkv(cur, 0, dch)
            for i in range(N):
                nxt = None
                if i + 1 < N:
                    nxt = {"x": load_x(i + 1)}
                    ln1(nxt)
                attn(cur, i % NCH, dch)
                projx(cur, i % NCH, dch)
                ln2p(cur)
                if nxt is not None:
                    front_qkv(nxt, (i + 1) % NCH, dch)
                mlp1(cur, i % NCH, dch)
                mlp2(cur, i)
                cur = nxt

    # Route the act-table pass through the filtered table list.
    orig = bacc_mod.get_activation_tables
    bacc_mod.get_activation_tables = _patched_tables
    try:
        nc.compile()
    finally:
        bacc_mod.get_activation_tables = orig
    return nc


def _prep_inputs(x, wq, wk, wv, wp, bp, w1, w2, g1, b1, g2, b2):
    """Host-side weight folding + per-core input maps."""
    f32 = np.float32
    qdt = ml_dtypes.float8_e4m3 if F8 else ml_dtypes.bfloat16
    adt = ml_dtypes.float8_e4m3 if F8A else ml_dtypes.bfloat16
    scale = HD ** -0.5
    wq_m = np.ascontiguousarray(np.asarray(wq).transpose(1, 0, 2).reshape(C, C))
    wk_m = np.ascontiguousarray(np.asarray(wk).transpose(1, 0, 2).reshape(C, C))
    wv_m = np.ascontiguousarray(np.asarray(wv).transpose(1, 0, 2).reshape(C, C))
    wq_p = (g1[:, None] * wq_m * scale).astype(f32)
    cq = (b1 @ wq_m * scale).astype(f32)
    wk_p = (g1[:, None] * wk_m).astype(f32)
    ck = (b1 @ wk_m).astype(f32)
    wv_p = (g1[:, None] * wv_m).astype(f32)
    cv = (b1 @ wv_m).astype(f32)
    w1_p = (g2[:, None] * np.asarray(w1)).astype(f32)
    c1 = (b2 @ w1).astype(f32)
    bp_p = (np.asarray(bp, f32) + cv @ np.asarray(wp, f32)).astype(f32)

    def ktile(w, kt):   # [K, M] -> [128, kt*M] k-tile-blocked, quantized
        K, M = w.shape
        return np.ascontiguousarray(
            (w * S).reshape(kt, 128, M).transpose(1, 0, 2).reshape(128, kt * M)
        ).astype(qdt)

    wq8 = ktile(wq_p, 3)
    wk8 = ktile(wk_p, 3)
    wv8 = ktile(wv_p, 3)
    wp8 = ktile(np.asarray(wp, f32), 3)
    w18 = ktile(w1_p, 3)
    w28 = ktile(np.asarray(w2, f32), 12)
    bpr = (S * bp_p)[None, :].astype(qdt)

    cqk = np.concatenate([cq.reshape(3, 128).T, ck.reshape(3, 128).T], axis=1)
    cqk = np.ascontiguousarray(cqk, dtype=f32)                       # [128, 6]
    c1t = np.ascontiguousarray((S * c1).reshape(12, 128).T, dtype=f32)
    idb = np.eye(128).astype(ml_dtypes.bfloat16)
    ii, jj = np.arange(128)[:, None], np.arange(128)[None, :]
    triT = np.where(ii > jj, 0.0, 1.0).astype(f32)    # mult-mask, s > t -> 0
    cmt = np.concatenate([triT, triT], axis=1)

    common = dict(wq=wq8, wk=wk8, wv=wv8, wp=wp8, w1=w18, w2=w28,
                  cqk=cqk, c1=c1t, bpr=bpr, idb=idb, cmt=cmt)
    in_maps = []
    for core in range(NCORES):
        xs = np.ascontiguousarray(
            np.asarray(x)[BL * core:BL * (core + 1)].reshape(TOK, C), dtype=f32)
        in_maps.append(dict(common, x=xs))
    return in_maps


def run(inputs, trace=False, trace_kwargs=None, debug=False):
    key = "nc_dbg" if debug else "nc"
    if key not in _CACHE:
        _CACHE[key] = _build(debug=debug)
    nc = _CACHE[key]
    in_maps = _prep_inputs(**inputs)
    res = run_bass_kernel_spmd(nc, in_maps, list(range(NCORES)),
                               trace=trace, **(trace_kwargs or {}))
    out = np.empty((B, T, C), dtype=np.float32)
    for core in range(NCORES):
        out[BL * core:BL * (core + 1)] = \
            res.results[core]["out"].reshape(BL, T, C)
    return out, res


def kernel(**inputs):
    out, _ = run(inputs)
    return out


# revision 10
# speedup vs baseline: 1.7845x; 1.6496x over previous
"""Trainium2 Bass kernel for a pre-LN causal-attention transformer block, v2.

Sharding: data-parallel over batch. B=64 -> 8 batches per NeuronCore,
processed in 4 chunks of 2 batches (512 tokens).

Design notes:
  - rstd = exp(-0.5*ln(var+eps)) on ACT, and the activation-table list fed to
    the table-load pass is filtered so Ln/Exp resolve to the combined
    natural_log_exp_and_others set -> one table load for the whole program.
  - Attention computed transposed: scoresT[s,t] = kT^T qT directly (kT
    stationary), two heads per score psum tile; causal mask added only on the
    diagonal blocks with a head-broadcast mask; exp writes probsT; attnV uses
    probsT as stationary with an appended ones-column on V giving row sums for
    free; normalization is a per-token reciprocal + broadcast multiply on the
    token-major o psum.
  - All six big GEMMs (Q,K,V,proj,MLP1,MLP2) run in fp8 e4m3 with DoubleRow
    perf mode. Weights are host-scaled by S=32 before quantization; the
    descale rides the psum->sbuf copies. attnV is fp8+DR too. The bp bias is
    a K=1 matmul row appended to the proj psum group.
  - Elementwise/copy work is spread across ACT/DVE/Pool (Pool = Q7 software,
    ~0.5x throughput: gets the lightest share).
"""
import numpy as np
import ml_dtypes

import concourse.tile as tile
import concourse.bacc as bacc_mod
from concourse import bacc, mybir
from concourse.bass_utils import run_bass_kernel_spmd
from concourse.hw_specs import get_activation_tables

F32 = mybir.dt.float32
BF16 = mybir.dt.bfloat16
FP8 = mybir.dt.float8e4
AF = mybir.ActivationFunctionType
ALU = mybir.AluOpType
PM = mybir.MatmulPerfMode

B, T, C = 64, 256, 384
H, HD = 6, 64
FF = 4 * C                      # 1536
NCORES = 8
BL = B // NCORES                # 8 batches per core
TOK = BL * T                    # 2048 tokens per core
CHB = 2                        # batches per chunk
NCH = BL // CHB                 # 4 chunks
CTOK = CHB * T                  # 512 tokens per chunk
NTT = CTOK // 128               # 4 token tiles per chunk
EPS = 1e-5
NEG = -30.0                     # additive causal mask value

F8 = True                       # fp8 + DoubleRow on the big GEMMs
F8A = True                      # fp8 + DoubleRow on attnV probs/V
S = 32.0 if F8 else 1.0         # host weight scale before fp8 quantization
WDT = FP8 if F8 else BF16       # dtype of quantized weights + fm activations
ADT = FP8 if F8A else BF16      # dtype of probs / V

# engine assignment knobs: 'a'=ACT, 'd'=DVE, 'p'=Pool
CFG = dict(
    tp_e0="add", tp_e1="a",       # transpose psum->sbuf copies (1024/512 cols)
    qk_eng="aa",                # qT, kT copies
    v_eng="a",                  # V copies
    mask_eng="d",             # per head-pair: p=affine_select, d=TT-mult
    onorm_eng="d",              # o normalize broadcast-mult
    norm_eng="pp",              # LN normalize halves (SBUF only: p ok)
    relu_eng="ada",              # MLP1 relu cycle (PSUM: a/d only)
    proj_eng="d",               # proj residual stt
    mlp2_eng="d",               # MLP2 residual stt
    pipeline=True,              # software-pipeline chunks
)

_CACHE = {}

_COMBINED_TABLE = "natural_log_exp_and_others"


def _patched_tables(arch):
    """Strip Ln/Exp from every act-func set except the combined one, so the
    table-load pass resolves both to a single resident table."""
    tabs = dict(get_activation_tables(arch))
    ln_exp = {AF.Ln, AF.Exp}
    out = {}
    for name, funcs in tabs.items():
        if name == _COMBINED_TABLE:
            out[name] = funcs
        else:
            out[name] = funcs - ln_exp
    return out


def _build(debug=False, repeat=1):
    nc = bacc.Bacc("TRN2", target_bir_lowering=False, debug=False,
                   num_devices=NCORES)

    # ---- DRAM I/O ----------------------------------------------------------
    x_d = nc.dram_tensor("x", [TOK, C], F32, kind="ExternalInput").ap()
    wq_d = nc.dram_tensor("wq", [128, 3 * C], WDT, kind="ExternalInput").ap()
    wk_d = nc.dram_tensor("wk", [128, 3 * C], WDT, kind="ExternalInput").ap()
    wv_d = nc.dram_tensor("wv", [128, 3 * C], WDT, kind="ExternalInput").ap()
    wp_d = nc.dram_tensor("wp", [128, 3 * C], WDT, kind="ExternalInput").ap()
    w1_d = nc.dram_tensor("w1", [128, 3 * FF], WDT, kind="ExternalInput").ap()
    w2_d = nc.dram_tensor("w2", [128, 12 * C], WDT, kind="ExternalInput").ap()
    cqk_d = nc.dram_tensor("cqk", [128, 6], F32, kind="ExternalInput").ap()
    c1_d = nc.dram_tensor("c1", [128, 12], F32, kind="ExternalInput").ap()
    bpr_d = nc.dram_tensor("bpr", [1, C], WDT, kind="ExternalInput").ap()
    idb_d = nc.dram_tensor("idb", [128, 128], BF16, kind="ExternalInput").ap()
    cmt_d = nc.dram_tensor("cmt", [128, 4 * 128], F32, kind="ExternalInput").ap()
    out_d = nc.dram_tensor("out", [TOK, C], F32, kind="ExternalOutput").ap()
    dbg = {}
    if debug:
        for nm, shape, dt in [
            ("d_h1", [128, NTT, C], BF16), ("d_h1T", [128, 3, CTOK], WDT),
            ("d_qT", [128, 3, CTOK], BF16), ("d_kT", [128, 3, CTOK], BF16),
            ("d_v", [128, NTT, 512], ADT), ("d_pT", [128, 2, 384], ADT),
            ("d_osb", [128, NTT, C], BF16), ("d_x2", [128, NTT, C], F32),
            ("d_a1T", [128, 12, CTOK], WDT),
        ]:
            dbg[nm] = nc.dram_tensor(nm, shape, dt, kind="ExternalOutput").ap()

    with tile.TileContext(nc) as tc:
        with (
            tc.tile_pool(name="const", bufs=1) as cp,
            tc.tile_pool(name="io", bufs=2) as iop,
            tc.tile_pool(name="act", bufs=2) as ap_,
            tc.tile_pool(name="probs", bufs=4) as ptp,
            tc.tile_pool(name="small", bufs=2) as smp,
            tc.tile_pool(name="tpps", bufs=2, space="PSUM") as tpp,
            tc.tile_pool(name="mmps", bufs=2, space="PSUM") as mmp,
            tc.tile_pool(name="scps", bufs=2, space="PSUM") as scp,
        ):
            # ---- persistent weights / constants ---------------------------
            idb_s = cp.tile([128, 128], BF16)
            nc.sync.dma_start(idb_s[:], idb_d[:])
            eps_s = cp.tile([128, 1], F32)
            nc.gpsimd.memset(eps_s[:], EPS)
            one_s = cp.tile([1, 128], WDT)
            nc.gpsimd.memset(one_s[:], 1.0)
            x_first = iop.tile([128, NTT, C], F32, tag="x")
            nc.sync.dma_start(
                x_first[:],
                x_d[0:CTOK, :].rearrange("(t p) c -> p t c", p=128))
            wq_s = cp.tile([128, 3, C], WDT)
            wk_s = cp.tile([128, 3, C], WDT)
            nc.sync.dma_start(wq_s[:].rearrange("p k c -> p (k c)"), wq_d[:])
            nc.sync.dma_start(wk_s[:].rearrange("p k c -> p (k c)"), wk_d[:])
            cqk_s = cp.tile([128, 6], F32)
            nc.sync.dma_start(cqk_s[:], cqk_d[:])
            wv_s = cp.tile([128, 3, C], WDT)
            nc.sync.dma_start(wv_s[:].rearrange("p k c -> p (k c)"), wv_d[:])
            cmt_s = cp.tile([128, 4, 128], F32)
            nc.sync.dma_start(cmt_s[:].rearrange("p a b -> p (a b)"), cmt_d[:])
            wp_s = cp.tile([128, 3, C], WDT)
            nc.sync.dma_start(wp_s[:].rearrange("p k c -> p (k c)"), wp_d[:])
            bpr_s = cp.tile([1, C], WDT)
            nc.sync.dma_start(bpr_s[:], bpr_d[:])
            w1_s = cp.tile([128, 3, FF], WDT)
            nc.sync.dma_start(w1_s[:].rearrange("p k c -> p (k c)"), w1_d[:])
            c1_s = cp.tile([128, 12], F32)
            nc.sync.dma_start(c1_s[:], c1_d[:])
            w2_s = cp.tile([128, 12, C], WDT)
            nc.sync.dma_start(w2_s[:].rearrange("p k c -> p (k c)"), w2_d[:])

            def load_x(ch):
                base = (ch % NCH) * CTOK
                t = iop.tile([128, NTT, C], F32, tag="x")
                nc.sync.dma_start(
                    t[:], x_d[base:base + CTOK, :].rearrange(
                        "(t p) c -> p t c", p=128))
                return t

            ENG = {"a": nc.scalar, "d": nc.vector, "p": nc.gpsimd}

            def layernorm2(src, tag):
                """token-major LN -> normalized bf16 tile [128, NTT, C].

                stats via bn_stats/bn_aggr (DVE), rstd = exp(-0.5 ln(var+eps))
                on ACT, normalize on engines from CFG['norm_eng'].
                """
                st6 = smp.tile([128, NTT, 6], F32, tag=f"{tag}s6")
                msig = smp.tile([128, NTT, 2], F32, tag=f"{tag}ms")
                for tt in range(NTT):
                    nc.vector.bn_stats(st6[:, tt], src[:, tt])
                    nc.vector.bn_aggr(msig[:, tt], st6[:, tt])
                lnv = smp.tile([128, NTT], F32, tag=f"{tag}lv")
                nc.scalar.activation(lnv[:], msig[:, :, 1], AF.Ln,
                                     bias=eps_s[:], scale=1.0)
                rstd = smp.tile([128, NTT], F32, tag=f"{tag}rs")
                nc.scalar.activation(rstd[:], lnv[:], AF.Exp,
                                     bias=0.0, scale=-0.5)
                h = ap_.tile([128, NTT, C], BF16, tag=f"{tag}h")
                for tt in range(NTT):
                    eng = ENG[CFG["norm_eng"][tt % len(CFG["norm_eng"])]]
                    eng.tensor_scalar(
                        h[:, tt], src[:, tt],
                        msig[:, tt, 0:1], rstd[:, tt:tt + 1],
                        op0=ALU.subtract, op1=ALU.mult)
                return h

            TP_SITE = {"h1": 0, "o": 1, "h2": 2}

            def transpose_ft(h, dst_dt, tag):
                """[128, NTT, C] token-major -> [128, 3, CTOK] feature-major."""
                hT = ap_.tile([128, 3, CTOK], dst_dt, tag=f"{tag}T")
                site = TP_SITE[tag]
                e0 = ENG[CFG["tp_e0"][site % len(CFG["tp_e0"])]]
                e1 = ENG[CFG["tp_e1"][site % len(CFG["tp_e1"])]]

                def cpy(eng, dst, src):
                    if eng is nc.scalar:
                        eng.copy(dst, src)
                    else:
                        eng.tensor_copy(dst, src)

                tpA = tpp.tile([128, 2, CTOK], BF16, tag="tpA")
                n = 2 * NTT
                i = 0
                for c in range(2):
                    for tt in range(NTT):
                        nc.tensor.matmul(
                            tpA[:, c, 128 * tt:128 * (tt + 1)],
                            h[:, tt, 128 * c:128 * (c + 1)], idb_s[:],
                            is_transpose=True, start=(i == 0), stop=(i == n - 1))
                        i += 1
                cpy(e0, hT[:, 0:2, :], tpA[:])
                tpB = tpp.tile([128, 2, CTOK], BF16, tag="tpA")
                for tt in range(NTT):
                    nc.tensor.matmul(
                        tpB[:, 0, 128 * tt:128 * (tt + 1)],
                        h[:, tt, 256:384], idb_s[:],
                        is_transpose=True, start=(tt == 0), stop=(tt == NTT - 1))
                cpy(e1, hT[:, 2, :], tpB[:, 0, :])
                return hT

            def mm_k3(ps, lhs_of, rhs_of, fp8):
                """psum = sum_k lhsT_k rhs_k over 3 k-tiles (DR pair + plain),
                leaves the accumulation group open (stop on caller)."""
                if fp8:
                    nc.tensor.matmul(ps, lhs_of(0, 2), rhs_of(0, 2),
                                     start=True, stop=False,
                                     perf_mode=PM.DoubleRow)
                    return lambda stop, extra=None: nc.tensor.matmul(
                        ps, lhs_of(2, 3), rhs_of(2, 3),
                        start=False, stop=stop)
                else:
                    for c in range(2):
                        nc.tensor.matmul(ps, lhs_of(c, c + 1), rhs_of(c, c + 1),
                                         start=(c == 0), stop=False)
                    return lambda stop, extra=None: nc.tensor.matmul(
                        ps, lhs_of(2, 3), rhs_of(2, 3),
                        start=False, stop=stop)

            def sl(t, k0, k1, c0=None, c1=None):
                return t[:, k0, c0:c1] if k1 == k0 + 1 else t[:, k0:k1, c0:c1]

            def front_qkv(st, ch, debug_ch):
                """h1T transposes + Q/K/V projections for one chunk (S2)."""
                # ---- h1 (bf16) -> h1T (WDT, feature-major) -----------------
                h1 = st["h1"]
                h1T = transpose_ft(h1, WDT, "h1")
                if debug and ch == debug_ch:
                    nc.sync.dma_start(dbg["d_h1"][:], h1[:])
                    nc.sync.dma_start(dbg["d_h1T"][:], h1T[:])

                # ---- Q^T, K^T (bf16, bias folded, descale by 1/S) ----------
                qT = ap_.tile([128, 3, CTOK], BF16, tag="qT")
                kT = ap_.tile([128, 3, CTOK], BF16, tag="kT")
                for w_s, oT, bcol, eng in ((wq_s, qT, 0, CFG["qk_eng"][0]),
                                           (wk_s, kT, 3, CFG["qk_eng"][1])):
                    for m in range(3):
                        ps = mmp.tile([128, CTOK], F32, tag="mm")
                        fin = mm_k3(
                            ps[:],
                            lambda k0, k1: sl(w_s, k0, k1, 128 * m, 128 * (m + 1)),
                            lambda k0, k1: sl(h1T, k0, k1), F8)
                        fin(True)
                        if eng == "d":
                            nc.vector.tensor_scalar(
                                oT[:, m], ps[:], 1.0 / S,
                                cqk_s[:, bcol + m:bcol + m + 1],
                                op0=ALU.mult, op1=ALU.add)
                        else:
                            nc.scalar.activation(
                                oT[:, m], ps[:], AF.Identity,
                                bias=cqk_s[:, bcol + m:bcol + m + 1],
                                scale=1.0 / S)

                # ---- V (token-major, ADT, with ones cols for row sums) -----
                v_sb = ap_.tile([128, NTT, 512], ADT, tag="v")
                nc.gpsimd.memset(v_sb[:, :, 64::65], 1.0)

                def v_tile(tt):
                    ps = mmp.tile([128, CTOK], F32, tag="mm")
                    fin = mm_k3(
                        ps[:, 0:C],
                        lambda k0, k1: sl(h1T, k0, k1, 128 * tt, 128 * (tt + 1)),
                        lambda k0, k1: sl(wv_s, k0, k1), F8)
                    fin(True)
                    dst = v_sb[:, tt, 0:390].rearrange(
                        "p (h e) -> p h e", h=6)[:, :, 0:64]
                    src = ps[:, 0:C].rearrange("p (h e) -> p h e", h=6)
                    if CFG["v_eng"] == "a":
                        nc.scalar.activation(dst, src, AF.Identity,
                                             bias=0.0, scale=1.0 / S)
                    else:
                        ENG[CFG["v_eng"]].tensor_scalar_mul(dst, src, 1.0 / S)

                v_tile(0)
                v_tile(1)
                if debug and ch == debug_ch:
                    nc.sync.dma_start(dbg["d_qT"][:], qT[:])
                    nc.sync.dma_start(dbg["d_kT"][:], kT[:])
                st.update(qT=qT, kT=kT, v=v_sb, v_tile=v_tile)

            def attn(st, ch, debug_ch):
                """attention (transposed scores), per batch in chunk (S3).

                Two heads per score-psum tile (2 banks); post-exp causal mask;
                attnV issues after the pair's exp, software-pipelined."""
                qT, kT, v_sb, v_tile = st["qT"], st["kT"], st["v"], st["v_tile"]
                o_sb = ap_.tile([128, NTT, C], BF16, tag="osb")
                for bb in range(CHB):
                    t0 = 2 * bb
                    pTs = []
                    for hp in range(3):         # head pairs (2hp, 2hp+1)
                        sc = scp.tile([128, 2, CTOK], F32, tag="sc")
                        for i in range(2):
                            h = 2 * hp + i
                            qp = 64 * (h % 2)
                            qm = h // 2
                            nc.tensor.matmul(
                                sc[:, i, 0:256],
                                kT[qp:qp + 64, qm, 128 * t0:128 * (t0 + 1)],
                                qT[qp:qp + 64, qm, 128 * t0:128 * (t0 + 2)],
                                start=True, stop=False)
                            nc.tensor.matmul(
                                sc[:, i, 256:384],
                                kT[qp:qp + 64, qm,
                                   128 * (t0 + 1):128 * (t0 + 2)],
                                qT[qp:qp + 64, qm,
                                   128 * (t0 + 1):128 * (t0 + 2)],
                                start=False, stop=True)
                        pT = ptp.tile([128, 2, 512], ADT, tag="pT")
                        nc.scalar.activation(pT[:, :, 0:384], sc[:, :, 0:384],
                                             AF.Exp, bias=0.0, scale=1.0)
                        # causal mask: zero probs where s > t on the two
                        # diagonal blocks of both heads in one op -- the
                        # 512-col head stride makes all 4 diag sub-blocks a
                        # uniform stride-256 2D view
                        dvw = pT[:].rearrange(
                            "p i (a b) -> p (i a) b", b=128)[:, 0::2, :]
                        ENG[CFG["mask_eng"][hp % len(CFG["mask_eng"])]
                            ].tensor_tensor(dvw, dvw, cmt_s[:], op=ALU.mult)
                        pTs.append(pT)
                        if debug and ch == debug_ch and bb == 0 and hp == 0:
                            nc.sync.dma_start(dbg["d_pT"][:], pT[:, :, 0:384])

                    if bb == 0:
                        v_tile(2)
                        v_tile(3)
                    op0 = mmp.tile([128, CTOK], F32, tag="mm")
                    op1 = mmp.tile([128, CTOK], F32, tag="mm")
                    ov0 = op0[:, 0:390].rearrange("p (h e) -> p h e", h=6)
                    ov1 = op1[:, 0:390].rearrange("p (h e) -> p h e", h=6)
                    for h in range(H):
                        pT = pTs[h // 2][:, h % 2]
                        # attnV: t0 tile plain (K=s0), t1 tile DR (K=s0+s1)
                        nc.tensor.matmul(
                            ov0[:, h], pT[:, 0:128],
                            v_sb[:, t0, 65 * h:65 * (h + 1)],
                            start=(h == 0), stop=(h == H - 1))
                        if F8A:
                            nc.tensor.matmul(
                                ov1[:, h],
                                pT[:, 128:384].rearrange(
                                    "p (k t) -> p k t", k=2),
                                v_sb[:, t0:t0 + 2, 65 * h:65 * (h + 1)],
                                start=(h == 0), stop=(h == H - 1),
                                perf_mode=PM.DoubleRow)
                        else:
                            nc.tensor.matmul(
                                ov1[:, h], pT[:, 128:256],
                                v_sb[:, t0, 65 * h:65 * (h + 1)],
                                start=(h == 0), stop=False)
                            nc.tensor.matmul(
                                ov1[:, h], pT[:, 256:384],
                                v_sb[:, t0 + 1, 65 * h:65 * (h + 1)],
                                start=False, stop=(h == H - 1))

                    for tt, ov in ((t0, ov0), (t0 + 1, ov1)):
                        rec = smp.tile([128, 6], F32, tag="rec")
                        nc.vector.reciprocal(rec[:], ov[:, :, 64])
                        ENG[CFG["onorm_eng"]].tensor_tensor(
                            o_sb[:, tt].rearrange("p (h e) -> p h e", h=6),
                            ov[:, :, 0:64],
                            rec[:, :, None].broadcast_to([128, 6, 64]),
                            op=ALU.mult)

                if debug and ch == debug_ch:
                    nc.sync.dma_start(dbg["d_osb"][:], o_sb[:])
                st["o_sb"] = o_sb

            def projx(st, ch, debug_ch):
                """oT transpose + proj + residual + bp -> x2 (S4)."""
                x_sb, o_sb = st["x"], st["o_sb"]
                oT = transpose_ft(o_sb, WDT, "o")
                x2 = ap_.tile([128, NTT, C], F32, tag="x2")
                for tt in range(NTT):
                    ps = mmp.tile([128, CTOK], F32, tag="mm")
                    fin = mm_k3(
                        ps[:, 0:C],
                        lambda k0, k1: sl(oT, k0, k1, 128 * tt, 128 * (tt + 1)),
                        lambda k0, k1: sl(wp_s, k0, k1), F8)
                    fin(False)
                    nc.tensor.matmul(ps[:, 0:C], one_s[:], bpr_s[:],
                                     start=False, stop=True)
                    ENG[CFG["proj_eng"]].scalar_tensor_tensor(
                        x2[:, tt], ps[:, 0:C], 1.0 / S, x_sb[:, tt],
                        op0=ALU.mult, op1=ALU.add)
                if debug and ch == debug_ch:
                    nc.sync.dma_start(dbg["d_x2"][:], x2[:])

                st["x2"] = x2

            def ln2p(st):
                st["h2"] = layernorm2(st["x2"], "l2")

            def mlp1(st, ch, debug_ch):
                h2T = transpose_ft(st["h2"], WDT, "h2")
                a1T = ap_.tile([128, 12, CTOK], WDT, tag="a1T")
                for fm in range(12):
                    ps = mmp.tile([128, CTOK], F32, tag="mm")
                    fin = mm_k3(
                        ps[:],
                        lambda k0, k1: sl(w1_s, k0, k1, 128 * fm, 128 * (fm + 1)),
                        lambda k0, k1: sl(h2T, k0, k1), F8)
                    fin(True)
                    r = CFG["relu_eng"][fm % len(CFG["relu_eng"])]
                    if r == "a":
                        nc.scalar.activation(
                            a1T[:, fm], ps[:], AF.Relu,
                            bias=c1_s[:, fm:fm + 1], scale=1.0)
                    else:
                        ENG[r].tensor_scalar(
                            a1T[:, fm], ps[:], c1_s[:, fm:fm + 1], 0.0,
                            op0=ALU.add, op1=ALU.max)
                if debug and ch == debug_ch:
                    nc.sync.dma_start(dbg["d_a1T"][:], a1T[:])
                st["a1T"] = a1T

            def ln1(st):
                st["h1"] = layernorm2(st["x"], "l1")

            def mlp2(st, ch):
                base = (ch % NCH) * CTOK
                x2, a1T = st["x2"], st["a1T"]
                o_out = iop.tile([128, NTT, C], F32, tag="o")
                for tt in range(NTT):
                    ps = mmp.tile([128, CTOK], F32, tag="mm")
                    if F8:
                        for j in range(6):
                            nc.tensor.matmul(
                                ps[:, 0:C],
                                a1T[:, 2 * j:2 * j + 2, 128 * tt:128 * (tt + 1)],
                                w2_s[:, 2 * j:2 * j + 2, :],
                                start=(j == 0), stop=(j == 5),
                                perf_mode=PM.DoubleRow)
                    else:
                        for j in range(12):
                            nc.tensor.matmul(
                                ps[:, 0:C],
                                a1T[:, j, 128 * tt:128 * (tt + 1)],
                                w2_s[:, j, :],
                                start=(j == 0), stop=(j == 11))
                    ENG[CFG["mlp2_eng"]].scalar_tensor_tensor(
                        o_out[:, tt], ps[:, 0:C], 1.0 / (S * S), x2[:, tt],
                        op0=ALU.mult, op1=ALU.add)
                nc.sync.dma_start(
                    out_d[base:base + CTOK, :].rearrange("(t p) c -> p t c", p=128),
                    o_out[:])

            # ---- software-pipelined chunk loop -------------------------
            # steady state per iteration i (chunk ch = i):
            #   LN1(i+1) | ATTN(i) | PROJ(i) | LN2(i) | QKV(i+1) | MLP(i)
            # so chunk i+1's LN runs during chunk i's attention era and its
            # PE-heavy QKV fills the LN2(i) latency before MLP(i).
            N = NCH * repeat
            dch = 0
            cur = {"x": x_first}
            ln1(cur)
            front_qkv(cur, 0, dch)
            for i in range(N):
                nxt = None
                if i + 1 < N:
                    nxt = {"x": load_x(i + 1)}
                    ln1(nxt)
                attn(cur, i % NCH, dch)
                projx(cur, i % NCH, dch)
                ln2p(cur)
                if nxt is not None:
                    front_qkv(nxt, (i + 1) % NCH, dch)
                mlp1(cur, i % NCH, dch)
                mlp2(cur, i)
                cur = nxt

    # Route the act-table pass through the filtered table list.
    orig = bacc_mod.get_activation_tables
    bacc_mod.get_activation_tables = _patched_tables
    try:
        nc.compile()
    finally:
        bacc_mod.get_activation_tables = orig
    return nc


def _prep_inputs(x, wq, wk, wv, wp, bp, w1, w2, g1, b1, g2, b2):
    """Host-side weight folding + per-core input maps."""
    f32 = np.float32
    qdt = ml_dtypes.float8_e4m3 if F8 else ml_dtypes.bfloat16
    adt = ml_dtypes.float8_e4m3 if F8A else ml_dtypes.bfloat16
    scale = HD ** -0.5
    wq_m = np.ascontiguousarray(np.asarray(wq).transpose(1, 0, 2).reshape(C, C))
    wk_m = np.ascontiguousarray(np.asarray(wk).transpose(1, 0, 2).reshape(C, C))
    wv_m = np.ascontiguousarray(np.asarray(wv).transpose(1, 0, 2).reshape(C, C))
    wq_p = (g1[:, None] * wq_m * scale).astype(f32)
    cq = (b1 @ wq_m * scale).astype(f32)
    wk_p = (g1[:, None] * wk_m).astype(f32)
    ck = (b1 @ wk_m).astype(f32)
    wv_p = (g1[:, None] * wv_m).astype(f32)
    cv = (b1 @ wv_m).astype(f32)
    w1_p = (g2[:, None] * np.asarray(w1)).astype(f32)
    c1 = (b2 @ w1).astype(f32)
    bp_p = (np.asarray(bp, f32) + cv @ np.asarray(wp, f32)).astype(f32)

    def ktile(w, kt):   # [K, M] -> [128, kt*M] k-tile-blocked, quantized
        K, M = w.shape
        return np.ascontiguousarray(
            (w * S).reshape(kt, 128, M).transpose(1, 0, 2).reshape(128, kt * M)
        ).astype(qdt)

    wq8 = ktile(wq_p, 3)
    wk8 = ktile(wk_p, 3)
    wv8 = ktile(wv_p, 3)
    wp8 = ktile(np.asarray(wp, f32), 3)
    w18 = ktile(w1_p, 3)
    w28 = ktile(np.asarray(w2, f32), 12)
    bpr = (S * bp_p)[None, :].astype(qdt)

    cqk = np.concatenate([cq.reshape(3, 128).T, ck.reshape(3, 128).T], axis=1)
    cqk = np.ascontiguousarray(cqk, dtype=f32)                       # [128, 6]
    c1t = np.ascontiguousarray((S * c1).reshape(12, 128).T, dtype=f32)
    idb = np.eye(128).astype(ml_dtypes.bfloat16)
    ii, jj = np.arange(128)[:, None], np.arange(128)[None, :]
    triT = np.where(ii > jj, 0.0, 1.0).astype(f32)    # mult-mask, s > t -> 0
    cmt = np.concatenate([triT, triT, triT, triT], axis=1)

    common = dict(wq=wq8, wk=wk8, wv=wv8, wp=wp8, w1=w18, w2=w28,
                  cqk=cqk, c1=c1t, bpr=bpr, idb=idb, cmt=cmt)
    in_maps = []
    for core in range(NCORES):
        xs = np.ascontiguousarray(
            np.asarray(x)[BL * core:BL * (core + 1)].reshape(TOK, C), dtype=f32)
        in_maps.append(dict(common, x=xs))
    return in_maps


def run(inputs, trace=False, trace_kwargs=None, debug=False):
    key = "nc_dbg" if debug else "nc"
    if key not in _CACHE:
        _CACHE[key] = _build(debug=debug)
    nc = _CACHE[key]
    in_maps = _prep_inputs(**inputs)
    res = run_bass_kernel_spmd(nc, in_maps, list(range(NCORES)),
                               trace=trace, **(trace_kwargs or {}))
    out = np.empty((B, T, C), dtype=np.float32)
    for core in range(NCORES):
        out[BL * core:BL * (core + 1)] = \
            res.results[core]["out"].reshape(BL, T, C)
    return out, res


def kernel(**inputs):
    out, _ = run(inputs)
    return out
